# revision 8
# baseline (speedup 1.0000x reference)
"""Deformable-conv kernel for Trainium2: 8-core data-parallel over batch.

kernel(x, offset_w, offset_b, conv_w) -> [8, 128, 56, 56] float32.
Each NeuronCore processes one batch image:
  offset conv in true-F32 PE matmuls (the reference sampler is discontinuous
  at integer x-coords, so offsets need ~1e-7 accuracy to reproduce its
  floor/trunc decisions) -> pixel-partitioned offsets (PE transpose)
  -> index/bilinear-weight math (DVE) -> bf16 indirect-DMA gathers from a
  row-pair-interleaved padded map (interior pixels: one 512B descriptor per
  (pixel, tap) fetching the anti-diagonal [bot-left, top-right] corner pair,
  index shifted by the exact-integer-hit mask; edge tiles: one 1KB 4-corner
  descriptor) -> 2-term (interior) / 4-term (edge) blend (DVE) -> bf16 PE
  transpose -> 3x3/stride-3 conv as 9 accumulating bf16 matmuls (PSUM).
"""
import sys
for _p in ("/opt/trn_rl_repo", "/root/.axon_site/_ro/trn_rl_repo"):
    if _p not in sys.path:
        sys.path.append(_p)

from contextlib import ExitStack

import numpy as np
import ml_dtypes

import concourse.bass as bass
import concourse.bacc as bacc
import concourse.mybir as mybir
import concourse.tile as tile
from concourse.masks import make_identity
from concourse.bass_utils import run_bass_kernel_spmd
from concourse.bass_interp import get_hw_module

F32 = mybir.dt.float32
BF16 = mybir.dt.bfloat16
I32 = mybir.dt.int32
I16 = mybir.dt.int16
ALU = mybir.AluOpType
ACTF = mybir.ActivationFunctionType

H = W = 56
HP = 58
NPIX = H * W          # 3136
NPAD = 3200           # padded pixel count (25 tiles of 128)
NTILE = 25
NTAP = 9
C = 128
OUT = 128
XR_ROWS = 3540 * 2    # interleaved row-pair map: entry e -> rows 2e, 2e+1
# edge tiles: pixel cols j<=2 or j>=52 live here (clip/trunc can fire in x)
EDGE_T = (0, 23, 24)
INT_T0, INT_T1 = 1, 23  # interior tiles [1, 23)


def build_kernel(nc):
    d_xcp = nc.dram_tensor("xcp", [C, HP * HP], F32, kind="ExternalInput").ap()
    d_xr2 = nc.dram_tensor("xr2", [XR_ROWS, C], BF16, kind="ExternalInput").ap()
    d_offw = nc.dram_tensor("offw", [C, NTAP * 18], F32, kind="ExternalInput").ap()
    d_offb = nc.dram_tensor("offb", [18, 1], F32, kind="ExternalInput").ap()
    d_convw = nc.dram_tensor("convw", [C, NTAP * OUT], BF16, kind="ExternalInput").ap()
    d_base = nc.dram_tensor("base", [C, NTILE * 18], F32, kind="ExternalInput").ap()
    d_out = nc.dram_tensor("out", [OUT, NPAD], F32, kind="ExternalOutput").ap()

    with tile.TileContext(nc) as tc:
        emit(tc, d_xcp, d_xr2, d_offw, d_offb, d_convw, d_base, d_out)
    return nc


def emit(tc, d_xcp, d_xr2, d_offw, d_offb, d_convw, d_base, d_out):
    nc = tc.nc
    ctx = ExitStack()
    with ctx:
        consts = ctx.enter_context(tc.tile_pool(name="consts", bufs=1))
        sb = ctx.enter_context(tc.tile_pool(name="sb", bufs=1))
        gpool = ctx.enter_context(tc.tile_pool(name="gpool", bufs=2))
        xpool = ctx.enter_context(tc.tile_pool(name="xpool", bufs=6))
        rpool = ctx.enter_context(tc.tile_pool(name="rpool", bufs=2))
        opool = ctx.enter_context(tc.tile_pool(name="opool", bufs=2))
        psActx = ExitStack()
        psA = psActx.enter_context(tc.tile_pool(name="psA", bufs=2, space="PSUM"))

        # ---- A: loads ----
        xcp = consts.tile([C, HP * HP], F32)
        nc.sync.dma_start(xcp[:], d_xcp[:])
        offw = consts.tile([C, NTAP * 18], F32)
        nc.sync.dma_start(offw[:], d_offw[:])
        convw = consts.tile([C, NTAP * OUT], BF16)
        nc.sync.dma_start(convw[:], d_convw[:])
        offb = consts.tile([18, 1], F32)
        nc.sync.dma_start(offb[:], d_offb[:])
        base = consts.tile([C, NTILE * 18], F32)
        nc.sync.dma_start(base[:], d_base[:])
        ident = consts.tile([C, C], F32)
        make_identity(nc, ident[:])
        identb = consts.tile([C, C], BF16)
        nc.vector.tensor_copy(out=identb[:], in_=ident[:])

        # ---- B: offset conv (true F32 for decision-grade precision) ----
        # col-major output pixels: chunk c covers j in [8c, 8c+8), all i.
        off_sb = sb.tile([18, NPAD], F32)
        xcp3 = xcp[:].rearrange("p (y x) -> p y x", y=HP)
        for ch in range(7):
            ps = psA.tile([18, 448], F32, tag="psA")
            for tap in range(NTAP):
                ky, kx = tap // 3, tap % 3
                rhs = xcp3[:, ky:ky + 56, kx + 8 * ch: kx + 8 * ch + 8] \
                    .transpose([0, 2, 1])
                nc.tensor.matmul(
                    ps[:], offw[:, tap * 18:(tap + 1) * 18], rhs,
                    start=(tap == 0), stop=(tap == NTAP - 1))
            nc.scalar.activation(off_sb[:, 448 * ch:448 * (ch + 1)], ps[:],
                                 ACTF.Identity, bias=offb[:, :1], scale=1.0)
        nc.vector.memset(off_sb[:, NPIX:], 0.0)

        # ---- C: transpose offsets to pixel-partitioned ----
        offT = sb.tile([C, NTILE * 18], F32)
        for t in range(NTILE):
            pst = psA.tile([C, 18], F32, tag="psA")
            nc.tensor.transpose(pst[:], off_sb[:, t * C:(t + 1) * C],
                                ident[:18, :18])
            nc.scalar.activation(offT[:, t * 18:(t + 1) * 18], pst[:],
                                 ACTF.Copy)

        # ---- D: index + weight math ----
        # layout [128, 25*18]: col (t*18 + k), k in 0..8 = y taps, 9..17 = x taps
        w_lt = sb.tile([C, NTILE * NTAP], F32)
        w_rb = sb.tile([C, NTILE * NTAP], F32)
        w_lb = sb.tile([C, NTILE * NTAP], F32)
        w_rt = sb.tile([C, NTILE * NTAP], F32)
        s0f = sb.tile([C, NTILE * NTAP], F32)
        s1f = sb.tile([C, NTILE * NTAP], F32)
        idxf = sb.tile([C, NTILE * NTAP], F32)

        tmp = sb.tile([C, NTILE * 18], F32, tag="dtmp")      # p
        q = sb.tile([C, NTILE * 18], F32, tag="dtmp2")       # q = floor(p)
        qlt = sb.tile([C, NTILE * 18], F32, tag="dtmp3")
        qrb = sb.tile([C, NTILE * 18], F32, tag="dtmp4")
        pc = sb.tile([C, NTILE * 18], F32, tag="dtmp5")
        gA = sb.tile([C, NTILE * 18], F32, tag="dtmp6")      # 1 - f
        hh = sb.tile([C, NTILE * 18], F32, tag="dtmp7")      # 1 - (qrb - pc)
        t0 = sb.tile([C, NTILE * NTAP], F32, tag="dtmp8")
        t1 = sb.tile([C, NTILE * NTAP], F32, tag="dtmp9")
        tt = sb.tile([C, NTILE * NTAP], F32, tag="dtmp10")

        def Y(ap):  # y-axis slice of [128, 25*18] -> [128, 25, 9]
            return ap[:].rearrange("p (t k) -> p t k", k=18)[:, :, 0:9]

        def X(ap):
            return ap[:].rearrange("p (t k) -> p t k", k=18)[:, :, 9:18]

        def V9(ap):  # [128, 25*9] -> [128, 25, 9]
            return ap[:].rearrange("p (t k) -> p t k", k=9)

        # p = base + offT
        nc.vector.tensor_tensor(tmp[:], base[:], offT[:], op=ALU.add)
        # q = floor(p) = cvt(p) - (p < cvt(p)); exact for any cvt rounding mode
        ti = sb.tile([C, NTILE * 18], I32, tag="dti")
        nc.vector.tensor_copy(out=ti[:], in_=tmp[:])
        nc.vector.tensor_copy(out=q[:], in_=ti[:])
        nc.vector.tensor_tensor(qlt[:], tmp[:], q[:], op=ALU.is_lt)
        nc.vector.tensor_tensor(q[:], q[:], qlt[:], op=ALU.subtract)
        # qlt = clip(q, 0, 57)
        nc.vector.tensor_scalar(qlt[:], q[:], 0.0, 57.0, op0=ALU.max, op1=ALU.min)
        # qrb = clip(q + 1, 0, 57)
        nc.vector.tensor_scalar(qrb[:], q[:], 1.0, 0.0, op0=ALU.add, op1=ALU.max)
        nc.vector.tensor_scalar(qrb[:], qrb[:], 57.0, None, op0=ALU.min)
        # pc = clip(p, 0, 57)
        nc.vector.tensor_scalar(pc[:], tmp[:], 0.0, 57.0, op0=ALU.max, op1=ALU.min)
        # gA = 1 - (pc - qlt);  hh = 1 - (qrb - pc)
        nc.vector.tensor_tensor(gA[:], pc[:], qlt[:], op=ALU.subtract)
        nc.vector.tensor_scalar(gA[:], gA[:], -1.0, 1.0, op0=ALU.mult, op1=ALU.add)
        nc.vector.tensor_tensor(hh[:], qrb[:], pc[:], op=ALU.subtract)
        nc.vector.tensor_scalar(hh[:], hh[:], -1.0, 1.0, op0=ALU.mult, op1=ALU.add)
        # trunc factors (x axis): t0 = (gA_x >= 1), t1 = (hh_x >= 1)
        nc.vector.tensor_scalar(V9(t0), X(gA), 1.0, None, op0=ALU.is_ge)
        nc.vector.tensor_scalar(V9(t1), X(hh), 1.0, None, op0=ALU.is_ge)
        # weights
        nc.vector.tensor_tensor(V9(w_lt), Y(gA), V9(t0), op=ALU.mult)
        nc.vector.tensor_tensor(V9(w_rb), Y(hh), V9(t1), op=ALU.mult)
        nc.vector.tensor_tensor(V9(w_lb), Y(gA), X(hh), op=ALU.mult)
        nc.vector.tensor_tensor(V9(w_rt), Y(hh), X(gA), op=ALU.mult)
        # interior slot weights (anti-diagonal pair + exact-hit fold):
        #   s0 = w_lt + w_rt*(1-t0)   (slot0 = bl normally, lt at exact hit)
        #   s1 = w_lb + t0*(w_rt - w_lb)  (slot1 = tr normally, bl at hit)
        nc.vector.tensor_tensor(tt[:], t0[:], w_rt[:], op=ALU.mult)
        nc.vector.tensor_tensor(s0f[:], w_lt[:], w_rt[:], op=ALU.add)
        nc.vector.tensor_tensor(s0f[:], s0f[:], tt[:], op=ALU.subtract)
        nc.vector.tensor_tensor(s1f[:], w_rt[:], w_lb[:], op=ALU.subtract)
        nc.vector.tensor_tensor(s1f[:], s1f[:], t0[:], op=ALU.mult)
        nc.vector.tensor_tensor(s1f[:], s1f[:], w_lb[:], op=ALU.add)
        s0, s1 = s0f, s1f  # scalar operands must stay f32
        wltb, wrbb, wlbb, wrtb = w_lt, w_rb, w_lb, w_rt
        # gather base index: s = clip(q, -1, 57); e = sy*60 + sx + 61
        # idx rows (C-units) of xr2: edge tiles 2e; interior 2e + 1 - t0
        nc.vector.tensor_scalar(q[:], q[:], -1.0, 57.0, op0=ALU.max, op1=ALU.min)
        nc.vector.tensor_scalar(V9(idxf), Y(q), 120.0, 122.0,
                                op0=ALU.mult, op1=ALU.add)
        nc.vector.tensor_tensor(V9(idxf), V9(idxf), X(q), op=ALU.add)
        nc.vector.tensor_tensor(V9(idxf), V9(idxf), X(q), op=ALU.add)
        # interior tiles: idx += 1 - t0
        iv = idxf[:].rearrange("p (t k) -> p t k", k=9)[:, INT_T0:INT_T1, :]
        tv = t0[:].rearrange("p (t k) -> p t k", k=9)[:, INT_T0:INT_T1, :]
        nc.vector.tensor_scalar(iv, iv, 1.0, None, op0=ALU.add)
        nc.vector.tensor_tensor(iv, iv, tv, op=ALU.subtract)
        # int16 idx in (t, n)-major order per tap via wrap DMAs:
        #   widx[r, n*200 + 8t + k] = idx16[16k + r, t*9 + n]
        idx16 = sb.tile([C, NTILE * NTAP], I16)
        nc.vector.tensor_copy(out=idx16[:], in_=idxf[:])
        idx16b = sb.tile([C, NTILE * NTAP], I16)
        nc.vector.tensor_copy(
            out=idx16b[:].rearrange("p (n t) -> p t n", n=NTAP),
            in_=idx16[:].rearrange("p (t n) -> p t n", t=NTILE))
        widx = sb.tile([C, NTAP * 200], I16)
        for k in range(8):
            src_ap = idx16b[16 * k:16 * k + 16, :].rearrange(
                "p (m t) -> p m t", t=NTILE)
            dst_ap = widx[0:16, :].rearrange(
                "p (m t k) -> p m t k", m=NTAP, t=NTILE)[:, :, :, k]
            nc.sync.dma_start(dst_ap, src_ap)
        for g in range(1, 8):
            nc.sync.dma_start(widx[16 * g:16 * g + 16, :], widx[0:16, :])

        # ---- E: per-tap gather + blend + transpose + conv ----
        psActx.close()  # release phase-B/C PSUM banks
        psT = ctx.enter_context(tc.tile_pool(name="psT", bufs=2, space="PSUM"))
        psO = ctx.enter_context(tc.tile_pool(name="psO", bufs=1, space="PSUM"))
        accs = [psO.tile([OUT, 512], F32, tag=f"acc{ch}", name=f"acc{ch}")
                for ch in range(6)]
        out6 = sb.tile([OUT, C], F32)  # SBUF accumulator for pixels 3072:3200

        # xr2 viewed as overlapping runs: row i = elements [i*C, i*C + len)
        xr_pair = bass.AP(tensor=d_xr2.tensor, offset=0,
                          ap=[[C, XR_ROWS - 1], [1, 2 * C]])
        xr_quad = bass.AP(tensor=d_xr2.tensor, offset=0,
                          ap=[[C, XR_ROWS - 3], [1, 4 * C]])
        for tap in range(NTAP):
            g2 = gpool.tile([C, NTILE, 2 * C], BF16, tag="g2", name="g2")
            g4 = gpool.tile([C, 4, 4 * C], BF16, tag="g4", name="g4")
            # interior tiles 1..22: anti-diagonal pair, 1 desc/(pix,tap)
            for j0, nj in ((0, 1024), (1024, 1024), (2048, 768)):
                s0i = (128 + j0) // 16
                nc.gpsimd.dma_gather(
                    out_ap=g2[:, INT_T0 + j0 // 128: INT_T0 + (j0 + nj) // 128, :],
                    in_ap=xr_pair,
                    idxs_ap=widx[:, tap * 200 + s0i: tap * 200 + s0i + nj // 16],
                    num_idxs=nj, num_idxs_reg=nj,
                    elem_size=2 * C, elem_step=C)
            # edge tiles 0 / 23,24: 4-corner quad, 1 desc/(pix,tap)
            nc.gpsimd.dma_gather(
                out_ap=g4[:, 0:1, :], in_ap=xr_quad,
                idxs_ap=widx[:, tap * 200: tap * 200 + 8],
                num_idxs=128, num_idxs_reg=128,
                elem_size=4 * C, elem_step=C)
            nc.gpsimd.dma_gather(
                out_ap=g4[:, 1:3, :], in_ap=xr_quad,
                idxs_ap=widx[:, tap * 200 + 184: tap * 200 + 200],
                num_idxs=256, num_idxs_reg=256,
                elem_size=4 * C, elem_step=C)

            rhs = rpool.tile([C, NPAD], BF16, tag="rhs")
            for tq in range(7):  # quads of pixel-tiles
                ntq = 4 if tq < 6 else 1
                pst = psT.tile([C, 512], F32, tag="pstr")
                pstb = pst[:].bitcast(BF16)
                for k in range(ntq):
                    t = tq * 4 + k
                    wcol = slice(t * NTAP + tap, t * NTAP + tap + 1)
                    xo = xpool.tile([C, C], BF16, tag="xo")
                    if t in EDGE_T:
                        ei = 0 if t == 0 else t - 22
                        nc.vector.tensor_scalar(
                            xo[:], g4[:, ei, 0:C], wltb[:, wcol], None,
                            op0=ALU.mult)
                        nc.vector.scalar_tensor_tensor(
                            xo[:], g4[:, ei, C:2 * C], wrtb[:, wcol], xo[:],
                            op0=ALU.mult, op1=ALU.add)
                        nc.vector.scalar_tensor_tensor(
                            xo[:], g4[:, ei, 2 * C:3 * C], wlbb[:, wcol], xo[:],
                            op0=ALU.mult, op1=ALU.add)
                        nc.vector.scalar_tensor_tensor(
                            xo[:], g4[:, ei, 3 * C:4 * C], wrbb[:, wcol], xo[:],
                            op0=ALU.mult, op1=ALU.add)
                    else:
                        nc.vector.tensor_scalar(
                            xo[:], g2[:, t, 0:C], s0[:, wcol], None,
                            op0=ALU.mult)
                        nc.vector.scalar_tensor_tensor(
                            xo[:], g2[:, t, C:2 * C], s1[:, wcol], xo[:],
                            op0=ALU.mult, op1=ALU.add)
                    nc.tensor.transpose(pstb[:, k * C:(k + 1) * C], xo[:],
                                        identb[:])
                nc.scalar.activation(rhs[:, tq * 512: tq * 512 + ntq * C],
                                     pstb[:, :ntq * C], ACTF.Copy)

            for ch in range(6):
                nc.tensor.matmul(
                    accs[ch][:],
                    convw[:, tap * OUT:(tap + 1) * OUT],
                    rhs[:, 512 * ch: 512 * ch + 512],
                    start=(tap == 0), stop=(tap == NTAP - 1))
            ps6 = psT.tile([C, 512], F32, tag="pstr")
            nc.tensor.matmul(ps6[:, :C],
                             convw[:, tap * OUT:(tap + 1) * OUT],
                             rhs[:, 3072:3200],
                             start=True, stop=True)
            if tap == 0:
                nc.vector.tensor_copy(out=out6[:], in_=ps6[:, :C])
            else:
                nc.vector.tensor_tensor(out6[:], out6[:], ps6[:, :C],
                                        op=ALU.add)

        # ---- F: output ----
        for ch in range(6):
            ob = opool.tile([OUT, 512], F32, tag="ob")
            nc.scalar.activation(ob[:], accs[ch][:], ACTF.Copy)
            nc.sync.dma_start(d_out[:, 512 * ch:512 * ch + 512], ob[:])
        nc.sync.dma_start(d_out[:, 3072:3200], out6[:])


# ---------------- host-side input prep ----------------

def prep_core_inputs(xb, offset_w, offset_b, conv_w):
    """Build the per-core in_map from one batch image [C, H, W] + weights."""
    f32 = np.float32
    xb = np.asarray(xb, f32)
    xp = np.pad(xb, ((0, 0), (1, 1), (1, 1)))                   # [C, 58, 58]
    xcp = np.ascontiguousarray(xp.reshape(C, HP * HP))
    xr60 = np.pad(xp, ((0, 0), (1, 1), (1, 1)), mode="edge")    # [C, 60, 60]
    xr60 = xr60.transpose(1, 2, 0)                              # [60, 60, C]
    # interleaved row pairs: xr2[2*(y*60+x)] = xr60[y,x]; [.. +1] = xr60[y+1,x]
    xr2 = np.stack([xr60[:-1], xr60[1:]], axis=2)               # [59, 60, 2, C]
    xr2 = np.ascontiguousarray(xr2.reshape(XR_ROWS, C)).astype(ml_dtypes.bfloat16)

    offw = np.empty((C, NTAP * 18), f32)
    convw = np.empty((C, NTAP * OUT), f32)
    for tap in range(NTAP):
        ky, kx = tap // 3, tap % 3
        offw[:, tap * 18:(tap + 1) * 18] = np.asarray(offset_w, f32)[:, :, ky, kx].T
        convw[:, tap * OUT:(tap + 1) * OUT] = np.asarray(conv_w, f32)[:, :, ky, kx].T
    offb = np.asarray(offset_b, f32).reshape(18, 1)

    # base grid [128, 25*18]: partition p, col t*18+k -> pixel t*128+p (col-major)
    r = np.arange(-1, 2, dtype=f32)
    py_n, px_n = np.meshgrid(r, r, indexing="ij")
    pny, pnx = py_n.ravel(), px_n.ravel()
    gy = np.arange(1, 57, dtype=f32)
    p0y, p0x = np.meshgrid(gy, gy, indexing="ij")
    p0yc, p0xc = p0y.T.ravel(), p0x.T.ravel()      # col-major pixels
    base = np.empty((NPAD, 18), f32)
    base[:NPIX, :NTAP] = p0yc[:, None] + pny[None, :]
    base[:NPIX, NTAP:] = p0xc[:, None] + pnx[None, :]
    base[NPIX:, :NTAP] = 28.0 + pny[None, :]
    base[NPIX:, NTAP:] = 28.0 + pnx[None, :]
    base = np.ascontiguousarray(
        base.reshape(NTILE, C, 18).transpose(1, 0, 2).reshape(C, NTILE * 18))

    return {"xcp": xcp, "xr2": xr2, "offw": offw, "offb": offb,
            "convw": convw.astype(ml_dtypes.bfloat16), "base": base}


def postprocess(out_np):
    """[OUT, 3200] col-major -> [OUT, 56, 56]."""
    o = out_np[:, :NPIX].reshape(OUT, W, H).transpose(0, 2, 1)
    return np.ascontiguousarray(o)


# ---------------- entry point ----------------

N_CORES = 8
_cache = {}


def _build():
    if "nc" in _cache:
        return _cache["nc"]
    nc = bacc.Bacc("TRN2", target_bir_lowering=False, debug=False,
                   enable_asserts=True, num_devices=N_CORES)
    build_kernel(nc)
    nc.compile()
    nc.m = get_hw_module(nc.m)
    _cache["nc"] = nc
    return nc


def kernel(x, offset_w, offset_b, conv_w):
    x = np.asarray(x, np.float32)
    assert x.shape == (N_CORES, C, H, W), x.shape
    nc = _build()
    in_maps = [prep_core_inputs(x[b], offset_w, offset_b, conv_w)
               for b in range(N_CORES)]
    res = run_bass_kernel_spmd(nc, in_maps, core_ids=list(range(N_CORES)))
    outs = [postprocess(res.results[b]["out"]) for b in range(N_CORES)]
    return np.stack(outs).astype(np.float32)
